# revision 7
# baseline (speedup 1.0000x reference)
"""BiAttention (BiDAF trilinear attention) Trainium2 Bass kernel.

Problem: B=16, N=M=1024, D=128, fp32.
  sim[b,n,m] = q.w_q + mem.w_m + (q * w_qm) . mem   (trilinear similarity)
  masked by qmask[b,n]*mmask[b,m] with -1e30 penalty
  out1 = softmax_m(sim) @ mem                       (query->memory attention)
  out2 = broadcast(softmax_n(max_m sim) @ q)        (memory->query attention)

Sharding: data-parallel over batch, 2 batches per core on 8 cores.

Per-core algorithm (per batch), all in the "transposed" layout
simT[m, n] so the softmax denominator and the value matmul need no
transposes of the big e matrix:
  - q_bf  = bf16(q) * qmask      (gpsimd cast-DMA + mask multiply)
  - mem_bf = bf16(mem) (+ ones column for the row-sum trick)
  - qT, memT via PE transposes;  memTs = memT * w_qm[d]
  - simT[m-tile] = memTs[:,t,:].T @ qT   (PE, bf16, c[m] NOT included)
  - eT = exp(simT + bias_m) on ACT, bias_m = c[m] + (mmask[m]-1)*1e30
    (a[n] cancels in the row softmax; c[m] rides the per-partition bias)
  - out1[n-chunk] = eT-chunk.T @ [mem_bf | 1]  accumulated over m-tiles
    -> col 128 is the row-sum; divide by it during the PSUM->SBUF copy.
  - qmask==0 rows: q column was zeroed so e[:,n] == exp(bias_m) exactly;
    a rank-1 correction matmul (1-qmask) x (S0*W - SV) rewrites those
    rows to W = mean(mem) with row-sum S0, reproducing the reference's
    uniform-softmax fallback exactly.
  - row_max for the m->q path: max_m e = tile-tree max on DVE (bf16)
    + PE transposes of the partial-max tile + one strided reduce.
    m2q weights w[n] = maxe[n] * exp(a[n]) * qmask[n]  (== exp(row_max)
    up to the softmax shift); out2 = (w @ q)/sum(w), broadcast on PE.
"""

import os
import sys

sys.path.insert(0, "/opt/trn_rl_repo")

import numpy as np
import ml_dtypes

import concourse.bass as bass
import concourse.bacc as bacc
import concourse.tile as tile
from concourse import mybir
from concourse import bass_utils

F32 = mybir.dt.float32
BF16 = mybir.dt.bfloat16

B, N, M, D = 16, 1024, 1024, 128
NCORES = 8
BPC = B // NCORES  # batches per core
NT = N // 128      # n-chunks
MT = M // 128      # m-tiles
NEG = -1.0e30
VW = D + 4         # value-matmul rhs width: 128 mem cols + ones col + pad
ONES_COL = D       # index of the ones column in mem_bf

_CACHE = {}


def _off1(j):
    """PSUM free offset of out1 chunk j: 3 chunks of width VW per 2KB bank."""
    return (j // 3) * 512 + (j % 3) * VW


def _bcast_inner(ap, n):
    """Append a step-0 inner free dim of size n to an AP (broadcast read)."""
    return bass.AP(tensor=ap.tensor, offset=ap.offset, ap=list(ap.ap) + [[0, n]])


def _rep_outer(ap, n):
    """Insert a step-0 free dim of size n after the partition dim."""
    new = [list(ap.ap[0])] + [[0, n]] + [list(x) for x in ap.ap[1:]]
    return bass.AP(tensor=ap.tensor, offset=ap.offset, ap=new)


def biattention_tile_kernel(tc, ins, outs):
    nc = tc.nc
    q_dr, mem_dr, qmaskT_dr, mmaskT_dr, qrow_dr, ident_dr, wq_dr, wm_dr, wqm_dr, onesM_dr, ones1_dr, onesrow_dr = ins
    out1_dr, out2_dr = outs

    import contextlib
    ctx = contextlib.ExitStack()

    singles = ctx.enter_context(tc.tile_pool(name="singles", bufs=1))
    perb = ctx.enter_context(tc.tile_pool(name="perb", bufs=2))
    psim = ctx.enter_context(tc.tile_pool(name="psim", bufs=2, space="PSUM"))
    pout1 = ctx.enter_context(tc.tile_pool(name="pout1", bufs=1, space="PSUM"))
    ptiny = ctx.enter_context(tc.tile_pool(name="ptiny", bufs=1, space="PSUM"))

    # ---- constants ----
    s_ident = singles.tile([128, 128], BF16)
    nc.sync.dma_start(out=s_ident, in_=ident_dr)
    s_wq = singles.tile([128, 1], BF16)
    nc.sync.dma_start(out=s_wq, in_=wq_dr)
    s_wm = singles.tile([128, 1], BF16)
    nc.sync.dma_start(out=s_wm, in_=wm_dr)
    s_wqm = singles.tile([128, 1], F32)
    nc.sync.dma_start(out=s_wqm, in_=wqm_dr)
    s_onesM = singles.tile([128, 1], BF16)   # 1/M
    nc.sync.dma_start(out=s_onesM, in_=onesM_dr)
    s_ones1 = singles.tile([128, 1], BF16)   # 1.0
    nc.sync.dma_start(out=s_ones1, in_=ones1_dr)
    s_onesrow = singles.tile([1, 128], F32)  # 1.0 row
    nc.sync.dma_start(out=s_onesrow, in_=onesrow_dr)

    s_qmaskT = singles.tile([128, BPC * NT], F32)
    nc.sync.dma_start(out=s_qmaskT, in_=qmaskT_dr)
    s_mmaskT = singles.tile([128, BPC * MT], F32)
    nc.sync.dma_start(out=s_mmaskT, in_=mmaskT_dr)
    s_qrow = singles.tile([1, BPC * N], F32)
    nc.sync.dma_start(out=s_qrow, in_=qrow_dr.rearrange("(o b) n -> o (b n)", o=1))

    # penm = (mmask - 1) * 1e30  in column layout
    s_penm = singles.tile([128, BPC * MT], F32)
    nc.vector.tensor_scalar(out=s_penm, in0=s_mmaskT, scalar1=-NEG, scalar2=NEG,
                            op0=mybir.AluOpType.mult, op1=mybir.AluOpType.add)
    # (1 - qmask) as a bf16 row, for the rank-1 correction matmul
    s_qinv = singles.tile([1, BPC * N], BF16)
    nc.vector.tensor_scalar(out=s_qinv, in0=s_qrow, scalar1=-1.0, scalar2=1.0,
                            op0=mybir.AluOpType.mult, op1=mybir.AluOpType.add)

    for b in range(BPC):
        # ---- loads (cast to bf16 during DMA on gpsimd) ----
        q_raw = perb.tile([128, NT, 128], BF16, tag="q_raw")
        nc.gpsimd.dma_start(out=q_raw, in_=q_dr[b].rearrange("(t p) d -> p t d", p=128))
        mem_bf = perb.tile([128, MT, VW], BF16, tag="mem_bf")
        nc.vector.memset(mem_bf[:, :, D:VW], 1.0)
        nc.gpsimd.dma_start(out=mem_bf[:, :, 0:D],
                            in_=mem_dr[b].rearrange("(t p) d -> p t d", p=128))

        # ---- q_bf = q * qmask (zero out masked query rows) ----
        q_bf = perb.tile([128, NT, 128], BF16, tag="q_bf")
        qmb = s_qmaskT[:, b * NT:(b + 1) * NT]  # [128, NT]
        nc.vector.tensor_tensor(out=q_bf, in0=q_raw, in1=_bcast_inner(qmb, 128),
                                op=mybir.AluOpType.mult)

        # ---- transposes: qT, memT(+scaled) ----
        tpq = psim.tile([128, NT, 128], BF16, tag="sim", name="tpq")
        for t in range(NT):
            nc.tensor.transpose(tpq[:, t, :], q_bf[:, t, :], s_ident)
        qT = perb.tile([128, NT, 128], BF16, tag="qT")
        nc.vector.tensor_copy(qT, tpq)

        tpm = psim.tile([128, MT, 128], BF16, tag="sim", name="tpm")
        for t in range(MT):
            nc.tensor.transpose(tpm[:, t, :], mem_bf[:, t, 0:D], s_ident)
        memT = perb.tile([128, MT, 128], BF16, tag="memT")
        nc.vector.tensor_copy(memT, tpm)
        memTs = perb.tile([128, MT, 128], BF16, tag="memTs")
        nc.vector.tensor_scalar(out=memTs, in0=tpm, scalar1=s_wqm, scalar2=None,
                                op0=mybir.AluOpType.mult)

        # ---- tiny matmuls: a[n] and c[m] columns ----
        # tinyB layout (one PSUM bank): a cols 0:8, c cols 8:16, SV 16:148,
        # W 148:280, out2row 280:408, wsum 408:416
        tinyB = ptiny.tile([128, 512], F32, name="tinyB")
        for j in range(NT):
            nc.tensor.matmul(tinyB[:, j:j + 1], lhsT=qT[:, j, :], rhs=s_wq,
                             start=True, stop=True)
        for t in range(MT):
            nc.tensor.matmul(tinyB[:, NT + t:NT + t + 1], lhsT=memT[:, t, :], rhs=s_wm,
                             start=True, stop=True)

        # bias_m = c + penm ;  expb = exp(bias_m)
        s_bias = perb.tile([128, MT], F32, tag="bias")
        nc.vector.tensor_tensor(out=s_bias, in0=tinyB[:, NT:NT + MT],
                                in1=s_penm[:, b * MT:(b + 1) * MT],
                                op=mybir.AluOpType.add)
        s_expb = perb.tile([128, MT], BF16, tag="expb")
        nc.scalar.activation(s_expb, s_bias, mybir.ActivationFunctionType.Exp)

        # ---- SV / W rows for the masked-row correction ----
        for t in range(MT):
            nc.tensor.matmul(tinyB[0:1, 16:16 + VW], lhsT=s_expb[:, t:t + 1],
                             rhs=mem_bf[:, t, :], start=(t == 0), stop=(t == MT - 1))
        for t in range(MT):
            nc.tensor.matmul(tinyB[0:1, 148:148 + VW], lhsT=s_onesM,
                             rhs=mem_bf[:, t, :], start=(t == 0), stop=(t == MT - 1))
        s_SV = perb.tile([1, VW], F32, tag="sv")
        nc.vector.tensor_copy(s_SV, tinyB[0:1, 16:16 + VW])
        s_corr = perb.tile([1, VW], F32, tag="corr")
        # corr = S0 * W - SV ; S0 = SV[ones col]
        nc.vector.tensor_scalar(out=s_corr, in0=tinyB[0:1, 148:148 + VW],
                                scalar1=s_SV[0:1, ONES_COL:ONES_COL + 1], scalar2=None,
                                op0=mybir.AluOpType.mult)
        s_corr2 = perb.tile([1, VW], BF16, tag="corr2")
        nc.vector.tensor_tensor(out=s_corr2, in0=s_corr, in1=s_SV,
                                op=mybir.AluOpType.subtract)

        # ---- sim + exp + value matmuls, per m-tile ----
        eT = perb.tile([128, MT, 1024], BF16, tag="eT")
        po1 = pout1.tile([128, 1536], F32, name="po1")
        qTv = qT.rearrange("p t d -> p (t d)")
        for t in range(MT):
            ps = psim.tile([128, 1024], F32, tag="sim", name=f"ps{b}_{t}")
            nc.tensor.matmul(ps[:, 0:512], lhsT=memTs[:, t, :], rhs=qTv[:, 0:512],
                             start=True, stop=True)
            nc.tensor.matmul(ps[:, 512:1024], lhsT=memTs[:, t, :], rhs=qTv[:, 512:1024],
                             start=True, stop=True)
            nc.scalar.activation(eT[:, t, :], ps, mybir.ActivationFunctionType.Exp,
                                 bias=s_bias[:, t:t + 1])
        # value matmuls; each n-chunk j is one PSUM accumulation group,
        # closed by the rank-1 qmask==0 correction matmul
        for j in range(NT):
            for t in range(MT):
                nc.tensor.matmul(po1[:, _off1(j):_off1(j) + VW],
                                 lhsT=eT[:, t, j * 128:(j + 1) * 128],
                                 rhs=mem_bf[:, t, :],
                                 start=(t == 0), stop=False)
            nc.tensor.matmul(po1[:, _off1(j):_off1(j) + VW],
                             lhsT=s_qinv[0:1, b * N + j * 128:b * N + (j + 1) * 128],
                             rhs=s_corr2,
                             start=False, stop=True)

        # ---- out1 = po1 / rowsum ----
        out1_sb = perb.tile([128, NT, 128], F32, tag="out1")
        s_rs = perb.tile([128, NT], F32, tag="rs")
        for j in range(NT):
            nc.vector.reciprocal(s_rs[:, j:j + 1],
                                 po1[:, _off1(j) + ONES_COL:_off1(j) + ONES_COL + 1])
        for j in range(NT):
            eng = nc.vector if j % 2 == 0 else nc.scalar
            if j % 2 == 0:
                nc.vector.tensor_scalar(out=out1_sb[:, j, :],
                                        in0=po1[:, _off1(j):_off1(j) + D],
                                        scalar1=s_rs[:, j:j + 1], scalar2=None,
                                        op0=mybir.AluOpType.mult)
            else:
                nc.scalar.mul(out1_sb[:, j, :], po1[:, _off1(j):_off1(j) + D],
                              s_rs[:, j:j + 1])
        nc.sync.dma_start(out=out1_dr[b].rearrange("(t p) d -> p t d", p=128),
                          in_=out1_sb)

        # ---- m->q path: maxe over m via DVE tile-tree + PE transposes ----
        s1 = perb.tile([128, 4, 1024], BF16, tag="s1")
        nc.vector.tensor_tensor(out=s1, in0=eT[:, 0:4, :], in1=eT[:, 4:8, :],
                                op=mybir.AluOpType.max)
        s2 = perb.tile([128, 2, 1024], BF16, tag="s2")
        nc.vector.tensor_tensor(out=s2, in0=s1[:, 0:2, :], in1=s1[:, 2:4, :],
                                op=mybir.AluOpType.max)
        pm = perb.tile([128, 1024], BF16, tag="pm")
        nc.vector.tensor_tensor(out=pm, in0=s2[:, 0, :], in1=s2[:, 1, :],
                                op=mybir.AluOpType.max)
        pmT = psim.tile([128, NT, 128], BF16, tag="sim", name="pmT")
        for j in range(NT):
            nc.tensor.transpose(pmT[:, j, :], pm[:, j * 128:(j + 1) * 128], s_ident)
        s_maxe = perb.tile([128, NT], F32, tag="maxe")
        nc.vector.tensor_reduce(s_maxe, pmT, axis=mybir.AxisListType.X,
                                op=mybir.AluOpType.max)

        # w = maxe * exp(a) * qmask
        s_ea = perb.tile([128, NT], F32, tag="ea")
        nc.scalar.activation(s_ea, tinyB[:, 0:NT], mybir.ActivationFunctionType.Exp)
        s_eaq = perb.tile([128, NT], F32, tag="eaq")
        nc.vector.tensor_tensor(out=s_eaq, in0=s_ea, in1=qmb, op=mybir.AluOpType.mult)
        s_w = perb.tile([128, NT], BF16, tag="w")
        nc.vector.tensor_tensor(out=s_w, in0=s_maxe, in1=s_eaq, op=mybir.AluOpType.mult)

        # out2 = (w @ q) / sum(w)
        for j in range(NT):
            nc.tensor.matmul(tinyB[0:1, 280:280 + D], lhsT=s_w[:, j:j + 1],
                             rhs=q_bf[:, j, :], start=(j == 0), stop=(j == NT - 1))
        nc.tensor.matmul(tinyB[0:1, 408:408 + NT], lhsT=s_ones1, rhs=s_w,
                         start=True, stop=True)
        s_ws = perb.tile([1, 1], F32, tag="ws")
        nc.vector.tensor_reduce(s_ws, tinyB[0:1, 408:408 + NT],
                                axis=mybir.AxisListType.X, op=mybir.AluOpType.add)
        s_wr = perb.tile([1, 1], F32, tag="wr")
        nc.vector.reciprocal(s_wr, s_ws)
        s_o2r = perb.tile([1, 128], F32, tag="o2r")
        nc.vector.tensor_scalar(out=s_o2r, in0=tinyB[0:1, 280:280 + D],
                                scalar1=s_wr, scalar2=None, op0=mybir.AluOpType.mult)
        pb = psim.tile([128, 128], F32, tag="sim", name=f"pb{b}")
        nc.tensor.matmul(pb, lhsT=s_onesrow, rhs=s_o2r, start=True, stop=True)
        s_o2 = perb.tile([128, 128], F32, tag="o2")
        nc.vector.tensor_copy(s_o2, pb)
        nc.sync.dma_start(out=out2_dr[b].rearrange("(t p) d -> p t d", p=128),
                          in_=_rep_outer(s_o2[:, :], NT))

    ctx.close()


def build_nc():
    nc = bacc.Bacc("TRN2", target_bir_lowering=False, debug=False)
    ins = [
        nc.dram_tensor("q", [BPC, N, D], F32, kind="ExternalInput").ap(),
        nc.dram_tensor("mem", [BPC, M, D], F32, kind="ExternalInput").ap(),
        nc.dram_tensor("qmaskT", [128, BPC * NT], F32, kind="ExternalInput").ap(),
        nc.dram_tensor("mmaskT", [128, BPC * MT], F32, kind="ExternalInput").ap(),
        nc.dram_tensor("qrow", [BPC, N], F32, kind="ExternalInput").ap(),
        nc.dram_tensor("ident", [128, 128], BF16, kind="ExternalInput").ap(),
        nc.dram_tensor("wq", [128, 1], BF16, kind="ExternalInput").ap(),
        nc.dram_tensor("wm", [128, 1], BF16, kind="ExternalInput").ap(),
        nc.dram_tensor("wqm", [128, 1], F32, kind="ExternalInput").ap(),
        nc.dram_tensor("onesM", [128, 1], BF16, kind="ExternalInput").ap(),
        nc.dram_tensor("ones1", [128, 1], BF16, kind="ExternalInput").ap(),
        nc.dram_tensor("onesrow", [1, 128], F32, kind="ExternalInput").ap(),
    ]
    outs = [
        nc.dram_tensor("out1", [BPC, N, D], F32, kind="ExternalOutput").ap(),
        nc.dram_tensor("out2", [BPC, N, D], F32, kind="ExternalOutput").ap(),
    ]
    with tile.TileContext(nc) as tc:
        biattention_tile_kernel(tc, ins, outs)
    nc.compile()
    return nc


def make_in_maps(query, memory, w_q, w_m, w_qm, query_mask, memory_mask):
    bf = ml_dtypes.bfloat16
    consts = {
        "ident": np.eye(128, dtype=bf),
        "wq": np.asarray(w_q, np.float32).reshape(128, 1).astype(bf),
        "wm": np.asarray(w_m, np.float32).reshape(128, 1).astype(bf),
        "wqm": np.asarray(w_qm, np.float32).reshape(128, 1),
        "onesM": np.full((128, 1), 1.0 / M, dtype=bf),
        "ones1": np.ones((128, 1), dtype=bf),
        "onesrow": np.ones((1, 128), np.float32),
    }
    in_maps = []
    for c in range(NCORES):
        sl = slice(c * BPC, (c + 1) * BPC)
        qm = np.asarray(query_mask[sl], np.float32)   # [BPC, N]
        mm = np.asarray(memory_mask[sl], np.float32)  # [BPC, M]
        qmT = np.concatenate(
            [qm[b].reshape(NT, 128).T for b in range(BPC)], axis=1)  # [128, BPC*NT]
        mmT = np.concatenate(
            [mm[b].reshape(MT, 128).T for b in range(BPC)], axis=1)
        in_maps.append({
            "q": np.ascontiguousarray(query[sl], dtype=np.float32),
            "mem": np.ascontiguousarray(memory[sl], dtype=np.float32),
            "qmaskT": np.ascontiguousarray(qmT),
            "mmaskT": np.ascontiguousarray(mmT),
            "qrow": np.ascontiguousarray(qm),
            **consts,
        })
    return in_maps


def kernel(query, memory, w_q, w_m, w_qm, query_mask, memory_mask):
    if "nc" not in _CACHE:
        _CACHE["nc"] = build_nc()
    nc = _CACHE["nc"]
    in_maps = make_in_maps(query, memory, w_q, w_m, w_qm, query_mask, memory_mask)
    res = bass_utils.run_bass_kernel_spmd(
        nc, in_maps, core_ids=list(range(NCORES)),
        trace=False,
    )
    out1 = np.concatenate([res.results[c]["out1"] for c in range(NCORES)], axis=0)
    out2 = np.concatenate([res.results[c]["out2"] for c in range(NCORES)], axis=0)
    _CACHE["last_exec_time_ns"] = res.exec_time_ns
    return out1.reshape(B, N, D), out2.reshape(B, N, D)


# revision 21
# speedup vs baseline: 1.2027x; 1.2027x over previous
"""BiAttention (BiDAF trilinear attention) Trainium2 Bass kernel.

Problem: B=16, N=M=1024, D=128, fp32.
  sim[b,n,m] = q.w_q + mem.w_m + (q * w_qm) . mem   (trilinear similarity)
  masked by qmask[b,n]*mmask[b,m] with -1e30 penalty
  out1 = softmax_m(sim) @ mem                       (query->memory attention)
  out2 = broadcast(softmax_n(max_m sim) @ q)        (memory->query attention)

Sharding: data-parallel over batch, 2 batches per core on 8 cores.

Per-core algorithm (per batch), all in the "transposed" layout
simT[m, n] so the softmax denominator and the value matmul need no
transposes of the big e matrix:
  - q_bf  = bf16(q) * qmask      (gpsimd cast-DMA + mask multiply)
  - mem_bf = bf16(mem) (+ ones column for the row-sum trick)
  - qT, memT via PE transposes;  memTs = memT * w_qm[d]
  - simT[m-tile] = memTs[:,t,:].T @ qT   (PE, bf16, c[m] NOT included)
  - eT = exp(simT + bias_m) on ACT, bias_m = c[m] + (mmask[m]-1)*1e30
    (a[n] cancels in the row softmax; c[m] rides the per-partition bias)
  - out1[n-chunk] = eT-chunk.T @ [mem_bf | 1]  accumulated over m-tiles
    -> col 128 is the row-sum; divide by it during the PSUM->SBUF copy.
  - qmask==0 rows: q column was zeroed so e[:,n] == exp(bias_m) exactly;
    a rank-1 correction matmul (1-qmask) x (S0*W - SV) rewrites those
    rows to W = mean(mem) with row-sum S0, reproducing the reference's
    uniform-softmax fallback exactly.
  - row_max for the m->q path: max_m e = tile-tree max on DVE (bf16)
    + PE transposes of the partial-max tile + one strided reduce.
    m2q weights w[n] = maxe[n] * exp(a[n]) * qmask[n]  (== exp(row_max)
    up to the softmax shift); out2 = (w @ q)/sum(w), broadcast on PE.
"""

import os
import sys

sys.path.insert(0, "/opt/trn_rl_repo")

import numpy as np
import ml_dtypes

import concourse.bass as bass
import concourse.bacc as bacc
import concourse.tile as tile
from concourse import mybir
from concourse import bass_isa
from concourse import bass_utils

F32 = mybir.dt.float32
BF16 = mybir.dt.bfloat16
MUL = mybir.AluOpType.mult
ADD = mybir.AluOpType.add
SUB = mybir.AluOpType.subtract
MAX = mybir.AluOpType.max
DIV = mybir.AluOpType.divide
EXP = mybir.ActivationFunctionType.Exp

B, N, M, D = 16, 1024, 1024, 128
NCORES = 8
BPC = B // NCORES  # batches per core
NT = N // 128      # n-chunks
MT = M // 128      # m-tiles
NEG = -1.0e30
VW = D + 4         # value-matmul rhs width: 128 mem cols + ones col + pad
ONES_COL = D       # index of the ones column in mem_bf

# bf16 const blob columns
CB_IDENT = 0       # [128, 128] identity
CB_WQ = 128
CB_WM = 129
CB_ONESM = 130     # 1/M
CB_ONES1 = 131     # 1.0
CBW = 132
# f32 const blob columns
CF_WQM = 0
CF_QMT = 1                 # qmaskT [128, BPC*NT]
CF_MMT = CF_QMT + BPC * 8  # mmaskT
CF_ONESROW = CF_MMT + BPC * 8  # ones row at partition 0, 128 cols
CF_ONE1C = CF_ONESROW + 128    # 1.0 column (all partitions)
CF_R1024 = CF_ONE1C + 1        # 1/1024 row at partition 0, 128 cols
CFW = CF_R1024 + 128

_CACHE = {}


def _off1(j):
    """PSUM free offset of out1 chunk j: 3 chunks of width VW per 2KB bank."""
    return (j // 3) * 512 + (j % 3) * VW


def _bcast_inner(ap, n):
    """Append a step-0 inner free dim of size n to an AP (broadcast read)."""
    return bass.AP(tensor=ap.tensor, offset=ap.offset, ap=list(ap.ap) + [[0, n]])


def _rep_outer(ap, n):
    """Insert a step-0 free dim of size n after the partition dim."""
    new = [list(ap.ap[0])] + [[0, n]] + [list(x) for x in ap.ap[1:]]
    return bass.AP(tensor=ap.tensor, offset=ap.offset, ap=new)


def _strided(ap1, step, count):
    """[P, 1] AP -> [P, count] with the given free step (elements)."""
    return bass.AP(tensor=ap1.tensor, offset=ap1.offset,
                   ap=[list(ap1.ap[0]), [step, count]])


def biattention_tile_kernel(tc, ins, outs):
    nc = tc.nc
    q_dr, mem_dr, cbf_dr, cf32_dr, qrow_dr = ins
    out1_dr, out2_dr = outs

    import contextlib
    ctx = contextlib.ExitStack()

    singles = ctx.enter_context(tc.tile_pool(name="singles", bufs=1))
    perb = ctx.enter_context(tc.tile_pool(name="perb", bufs=2))
    psim = ctx.enter_context(tc.tile_pool(name="psim", bufs=2, space="PSUM"))
    pout1 = ctx.enter_context(tc.tile_pool(name="pout1", bufs=1, space="PSUM"))
    ptiny = ctx.enter_context(tc.tile_pool(name="ptiny", bufs=1, space="PSUM"))

    # ---- constants (packed: 3 DMAs total) ----
    cb = singles.tile([128, CBW], BF16)
    nc.sync.dma_start(out=cb, in_=cbf_dr)
    cf = singles.tile([128, CFW], F32)
    nc.sync.dma_start(out=cf, in_=cf32_dr)
    s_qrow = singles.tile([1, BPC * N], F32)
    nc.sync.dma_start(out=s_qrow, in_=qrow_dr.rearrange("(o b) n -> o (b n)", o=1))

    s_ident = cb[:, CB_IDENT:CB_IDENT + 128]
    s_wq = cb[:, CB_WQ:CB_WQ + 1]
    s_wm = cb[:, CB_WM:CB_WM + 1]
    s_onesM = cb[:, CB_ONESM:CB_ONESM + 1]
    s_ones1 = cb[:, CB_ONES1:CB_ONES1 + 1]
    s_wqm = cf[:, CF_WQM:CF_WQM + 1]
    s_qmaskT = cf[:, CF_QMT:CF_QMT + BPC * NT]
    s_mmaskT = cf[:, CF_MMT:CF_MMT + BPC * MT]
    s_onesrow = cf[0:1, CF_ONESROW:CF_ONESROW + 128]
    s_onef = cf[0:1, CF_ONESROW:CF_ONESROW + 1]   # scalar 1.0 f32
    s_onecol = cf[:, CF_ONE1C:CF_ONE1C + 1]       # 1.0 on all partitions

    # dummy exp to pull the ACT table load off the critical path
    s_dummy = singles.tile([1, 1], F32)
    nc.scalar.activation(s_dummy, s_onef, EXP)

    s_penm = singles.tile([128, BPC * MT], F32)
    s_qinv = singles.tile([1, BPC * N], BF16)

    s_r1024 = cf[0:1, CF_R1024:CF_R1024 + 128]

    # tinyB: one PSUM bank shared by both batches (disjoint 256-word halves)
    # per-batch regions (relative to b*256): a 0:8, c 8:16, corr 16:148,
    # S0row 148:156, S0bcast 156:157
    tinyB = ptiny.tile([128, 512], F32, name="tinyB")

    st = [dict() for _ in range(BPC)]

    def prologue(b):
        v = st[b]
        base = b * 256
        q_raw = perb.tile([128, NT, 128], BF16, tag="q_raw", name=f"q_raw{b}")
        nc.gpsimd.dma_start(out=q_raw, in_=q_dr[b].rearrange("(t p) d -> p t d", p=128))
        mem_bf = perb.tile([128, MT, VW], BF16, tag="mem_bf", name=f"mem_bf{b}")
        nc.vector.memset(mem_bf[:, :, D:VW], 1.0)
        nc.gpsimd.dma_start(out=mem_bf[:, :, 0:D],
                            in_=mem_dr[b].rearrange("(t p) d -> p t d", p=128))
        v["mem_bf"] = mem_bf

        # q_bf = q * qmask, per chunk so transposes can start early
        q_bf = perb.tile([128, NT, 128], BF16, tag="q_bf", name=f"q_bf{b}")
        qmb = s_qmaskT[:, b * NT:(b + 1) * NT]
        v["qmb"] = qmb
        nc.vector.tensor_tensor(out=q_bf, in0=q_raw, in1=_bcast_inner(qmb, 128),
                                op=MUL)
        v["q_bf"] = q_bf

        # transposes into the (currently idle) out1 PSUM banks
        tp = pout1.tile([128, 2 * NT, 128], BF16, tag="po1", name=f"tp{b}")
        for t in range(NT):
            nc.tensor.transpose(tp[:, t, :], q_bf[:, t, :], s_ident)
        for t in range(MT):
            nc.tensor.transpose(tp[:, NT + t, :], mem_bf[:, t, 0:D], s_ident)
        qT = perb.tile([128, NT, 128], BF16, tag="qT", name=f"qT{b}")
        nc.vector.tensor_copy(qT, tp[:, 0:NT, :])
        v["qT"] = qT
        memT = perb.tile([128, MT, 128], BF16, tag="memT", name=f"memT{b}")
        memTs = perb.tile([128, MT, 128], BF16, tag="memTs", name=f"memTs{b}")
        nc.vector.tensor_scalar(out=memTs, in0=tp[:, NT:NT + MT, :],
                                scalar1=s_wqm, scalar2=None, op0=MUL)
        nc.vector.tensor_copy(memT, tp[:, NT:NT + MT, :])
        v["memTs"] = memTs

        # a[n], c[m] columns
        for j in range(NT):
            nc.tensor.matmul(tinyB[:, base + j:base + j + 1], lhsT=qT[:, j, :],
                             rhs=s_wq, start=True, stop=True)
        for t in range(MT):
            nc.tensor.matmul(tinyB[:, base + NT + t:base + NT + t + 1],
                             lhsT=memT[:, t, :], rhs=s_wm, start=True, stop=True)

        # bias_m = c + penm ; expb = exp(bias_m)
        if b == 0:
            nc.vector.tensor_scalar(out=s_penm, in0=s_mmaskT, scalar1=-NEG,
                                    scalar2=NEG, op0=MUL, op1=ADD)
        s_bias = perb.tile([128, MT], F32, tag="bias", name=f"bias{b}")
        nc.vector.tensor_tensor(out=s_bias, in0=tinyB[:, base + NT:base + NT + MT],
                                in1=s_penm[:, b * MT:(b + 1) * MT], op=ADD)
        v["bias"] = s_bias
        # early pieces of the masked-row correction: expb = exp(bias),
        # S0 = sum(expb) via DVE partial + gpsimd partition all-reduce,
        # cl = S0/M - expb  (the PE corr matmuls run later, in corrblock)
        s_expb = perb.tile([128, MT], BF16, tag="expb", name=f"expb{b}")
        nc.scalar.activation(s_expb, s_bias, EXP)
        v["expb"] = s_expb
        s_S0p = perb.tile([128, 1], F32, tag="s0p", name=f"s0p{b}")
        nc.vector.tensor_reduce(s_S0p, s_expb, axis=mybir.AxisListType.X, op=ADD)
        s_S0c = perb.tile([128, 1], F32, tag="s0c", name=f"s0c{b}")
        nc.gpsimd.partition_all_reduce(s_S0c, s_S0p, channels=128,
                                       reduce_op=bass_isa.ReduceOp.add)
        s_cl = perb.tile([128, MT], BF16, tag="cl", name=f"cl{b}")
        nc.vector.scalar_tensor_tensor(out=s_cl, in0=_bcast_inner(s_S0c, MT),
                                       scalar=1.0 / M, in1=s_expb,
                                       op0=MUL, op1=SUB)
        v["cl"] = s_cl


    def corrblock(b):
        v = st[b]
        base = b * 256
        mem_bf = v["mem_bf"]
        if b == 0:
            nc.vector.tensor_scalar(out=s_qinv, in0=s_qrow, scalar1=-1.0,
                                    scalar2=1.0, op0=MUL, op1=ADD)
        s_cl = v["cl"]
        for t in range(MT):
            nc.tensor.matmul(tinyB[0:1, base + 16:base + 16 + VW],
                             lhsT=s_cl[:, t:t + 1], rhs=mem_bf[:, t, :],
                             start=(t == 0), stop=(t == MT - 1))
        s_corr2 = perb.tile([1, VW], BF16, tag="corr2", name=f"corr2{b}")
        nc.vector.tensor_copy(s_corr2, tinyB[0:1, base + 16:base + 16 + VW])
        v["corr2"] = s_corr2

    def valgroup(b, j, po1):
        v = st[b]
        eT, mem_bf = v["eT"], v["mem_bf"]
        for t in range(MT):
            nc.tensor.matmul(po1[:, _off1(j):_off1(j) + VW],
                             lhsT=eT[:, t, j * 128:(j + 1) * 128],
                             rhs=mem_bf[:, t, :],
                             start=(t == 0), stop=False)
        nc.tensor.matmul(po1[:, _off1(j):_off1(j) + VW],
                         lhsT=s_qinv[0:1, b * N + j * 128:b * N + (j + 1) * 128],
                         rhs=v["corr2"], start=False, stop=True)

    def fused(be, bv):
        """Interleave batch be's sim+exp+running-max with bv's value groups."""
        po1 = None
        if bv is not None:
            po1 = pout1.tile([128, 1536], F32, tag="po1", name=f"po1_{bv}")
            st[bv]["po1"] = po1
        if be is not None:
            v = st[be]
            eT = perb.tile([128, MT, 1024], BF16, tag="eT", name=f"eT{be}")
            v["eT"] = eT
            pm = perb.tile([128, 1024], BF16, tag="pm", name=f"pm{be}")
            v["pm"] = pm
            qTv = v["qT"].rearrange("p t d -> p (t d)")
            memTs = v["memTs"]
        for k in range(MT):
            if be is not None:
                ps = psim.tile([128, 1024], F32, tag="sim", name=f"ps{be}_{k}")
                nc.tensor.matmul(ps[:, 0:512], lhsT=memTs[:, k, :],
                                 rhs=qTv[:, 0:512], start=True, stop=True)
                nc.tensor.matmul(ps[:, 512:1024], lhsT=memTs[:, k, :],
                                 rhs=qTv[:, 512:1024], start=True, stop=True)
                nc.scalar.activation(eT[:, k, :], ps, EXP,
                                     bias=v["bias"][:, k:k + 1])
                if k == 1:
                    nc.vector.tensor_tensor(out=pm, in0=eT[:, 0, :],
                                            in1=eT[:, 1, :], op=MAX)
                elif k > 1:
                    nc.vector.tensor_tensor(out=pm, in0=pm, in1=eT[:, k, :], op=MAX)
            if bv is not None:
                valgroup(bv, k, po1)
        if bv is not None:
            valfinish(bv)

    def valfinish(b):
        v = st[b]
        po1 = v["po1"]
        out1_sb = perb.tile([128, NT, 128], F32, tag="out1", name=f"out1_{b}")
        s_rsm = perb.tile([128, NT], F32, tag="rsm", name=f"rsm{b}")
        for g, cnt in ((0, 3), (1, 3), (2, 2)):
            src = _strided(po1[:, g * 512 + ONES_COL:g * 512 + ONES_COL + 1], VW, cnt)
            nc.vector.tensor_copy(s_rsm[:, g * 3:g * 3 + cnt], src)
        s_rsi = perb.tile([128, NT], F32, tag="rsi", name=f"rsi{b}")
        nc.vector.reciprocal_approx_fast(s_rsi, s_rsm)
        for j in range(NT):
            eng_dve = (j % 2 == 0)
            if eng_dve:
                nc.vector.tensor_scalar(out=out1_sb[:, j, :],
                                        in0=po1[:, _off1(j):_off1(j) + D],
                                        scalar1=s_rsi[:, j:j + 1], scalar2=None,
                                        op0=MUL)
            else:
                nc.scalar.mul(out1_sb[:, j, :], po1[:, _off1(j):_off1(j) + D],
                              s_rsi[:, j:j + 1])
        h = NT // 2
        dst = out1_dr[b].rearrange("(t p) d -> p t d", p=128)
        nc.sync.dma_start(out=dst[:, 0:h, :], in_=out1_sb[:, 0:h, :])
        nc.sync.dma_start(out=dst[:, h:NT, :], in_=out1_sb[:, h:NT, :])

    def taila(b):
        v = st[b]
        pm = v["pm"]
        pmT = psim.tile([128, NT, 128], BF16, tag="sim", name=f"pmT{b}")
        for j in range(NT):
            nc.tensor.transpose(pmT[:, j, :], pm[:, j * 128:(j + 1) * 128], s_ident)
        s_maxe = perb.tile([128, NT], F32, tag="maxe", name=f"maxe{b}")
        nc.vector.tensor_reduce(s_maxe, pmT, axis=mybir.AxisListType.X, op=MAX)
        v["maxe"] = s_maxe
        s_ea = perb.tile([128, NT], F32, tag="ea", name=f"ea{b}")
        nc.scalar.activation(s_ea, tinyB[:, b * 256:b * 256 + NT], EXP)
        s_eaq = perb.tile([128, NT], F32, tag="eaq", name=f"eaq{b}")
        nc.vector.tensor_tensor(out=s_eaq, in0=s_ea, in1=v["qmb"], op=MUL)
        s_w = perb.tile([128, NT], BF16, tag="w", name=f"w{b}")
        nc.vector.tensor_tensor(out=s_w, in0=s_maxe, in1=s_eaq, op=MUL)
        v["w"] = s_w

    def tailb(b):
        v = st[b]
        s_w = v["w"]
        # out2 row, weight sum, broadcast -- all in one spare psim slot
        tb = psim.tile([128, 512], F32, tag="sim", name=f"tb{b}")
        for j in range(NT):
            nc.tensor.matmul(tb[0:1, 0:D], lhsT=s_w[:, j:j + 1],
                             rhs=v["q_bf"][:, j, :], start=(j == 0),
                             stop=(j == NT - 1))
        nc.tensor.matmul(tb[0:1, 136:136 + NT], lhsT=s_ones1, rhs=s_w,
                         start=True, stop=True)
        s_ws = perb.tile([1, 1], F32, tag="ws", name=f"ws{b}")
        nc.vector.tensor_reduce(s_ws, tb[0:1, 136:136 + NT],
                                axis=mybir.AxisListType.X, op=ADD)
        s_wsr = perb.tile([1, 1], F32, tag="wsr", name=f"wsr{b}")
        nc.vector.reciprocal_approx_fast(s_wsr, s_ws)
        s_o2r = perb.tile([1, 128], F32, tag="o2r", name=f"o2r{b}")
        nc.vector.tensor_scalar(out=s_o2r, in0=tb[0:1, 0:D],
                                scalar1=s_wsr, scalar2=None, op0=MUL)
        nc.tensor.matmul(tb[:, 256:384], lhsT=s_onesrow, rhs=s_o2r,
                         start=True, stop=True)
        s_o2 = perb.tile([128, 128], F32, tag="o2", name=f"o2{b}")
        nc.vector.tensor_copy(s_o2, tb[:, 256:384])
        nc.scalar.dma_start(out=out2_dr[b].rearrange("(t p) d -> p t d", p=128),
                          in_=_rep_outer(s_o2[:, :], NT))

    prologue(0)
    fused(0, None)
    corrblock(0)
    prologue(1)
    fused(1, 0)
    corrblock(1)
    taila(0)
    tailb(0)
    taila(1)
    fused(None, 1)
    tailb(1)

    ctx.close()


def build_nc():
    nc = bacc.Bacc("TRN2", target_bir_lowering=False, debug=False)
    ins = [
        nc.dram_tensor("q", [BPC, N, D], F32, kind="ExternalInput").ap(),
        nc.dram_tensor("mem", [BPC, M, D], F32, kind="ExternalInput").ap(),
        nc.dram_tensor("cbf", [128, CBW], BF16, kind="ExternalInput").ap(),
        nc.dram_tensor("cf32", [128, CFW], F32, kind="ExternalInput").ap(),
        nc.dram_tensor("qrow", [BPC, N], F32, kind="ExternalInput").ap(),
    ]
    outs = [
        nc.dram_tensor("out1", [BPC, N, D], F32, kind="ExternalOutput").ap(),
        nc.dram_tensor("out2", [BPC, N, D], F32, kind="ExternalOutput").ap(),
    ]
    with tile.TileContext(nc) as tc:
        biattention_tile_kernel(tc, ins, outs)
    nc.compile()
    return nc


def make_in_maps(query, memory, w_q, w_m, w_qm, query_mask, memory_mask):
    bf = ml_dtypes.bfloat16
    cbf = np.zeros((128, CBW), dtype=bf)
    cbf[:, 0:128] = np.eye(128, dtype=bf)
    cbf[:, CB_WQ] = np.asarray(w_q, np.float32).astype(bf)
    cbf[:, CB_WM] = np.asarray(w_m, np.float32).astype(bf)
    cbf[:, CB_ONESM] = bf(1.0 / M)
    cbf[:, CB_ONES1] = bf(1.0)
    in_maps = []
    for c in range(NCORES):
        sl = slice(c * BPC, (c + 1) * BPC)
        qm = np.asarray(query_mask[sl], np.float32)   # [BPC, N]
        mm = np.asarray(memory_mask[sl], np.float32)  # [BPC, M]
        cf32 = np.zeros((128, CFW), dtype=np.float32)
        cf32[:, CF_WQM] = np.asarray(w_qm, np.float32)
        for b in range(BPC):
            cf32[:, CF_QMT + b * NT:CF_QMT + (b + 1) * NT] = qm[b].reshape(NT, 128).T
            cf32[:, CF_MMT + b * MT:CF_MMT + (b + 1) * MT] = mm[b].reshape(MT, 128).T
        cf32[0, CF_ONESROW:CF_ONESROW + 128] = 1.0
        cf32[:, CF_ONE1C] = 1.0
        cf32[0, CF_R1024:CF_R1024 + 128] = 1.0 / M
        in_maps.append({
            "q": np.ascontiguousarray(query[sl], dtype=np.float32),
            "mem": np.ascontiguousarray(memory[sl], dtype=np.float32),
            "cbf": cbf,
            "cf32": cf32,
            "qrow": np.ascontiguousarray(qm),
        })
    return in_maps


def kernel(query, memory, w_q, w_m, w_qm, query_mask, memory_mask):
    if "nc" not in _CACHE:
        _CACHE["nc"] = build_nc()
    nc = _CACHE["nc"]
    in_maps = make_in_maps(query, memory, w_q, w_m, w_qm, query_mask, memory_mask)
    res = bass_utils.run_bass_kernel_spmd(
        nc, in_maps, core_ids=list(range(NCORES)), trace=False,
    )
    out1 = np.concatenate([res.results[c]["out1"] for c in range(NCORES)], axis=0)
    out2 = np.concatenate([res.results[c]["out2"] for c in range(NCORES)], axis=0)
    _CACHE["last_exec_time_ns"] = res.exec_time_ns
    return out1.reshape(B, N, D), out2.reshape(B, N, D)
